# revision 1
# baseline (speedup 1.0000x reference)
"""Cross-attention Trainium2 Bass kernel (8 NeuronCores, SPMD).

Problem: B=4, Sd=Se=2048, E=1024, H=16, D=64 cross-attention
  Q = dec @ Wq; K = enc @ Wk; V = enc @ Wv
  out = softmax(Q K^T / sqrt(D)) V @ Wo + b_o

Sharding (hardcoded): core c -> batch b=c//2, head-group g=c%2 (8 heads).
Each core gets transposed bf16 activations (dec[b].T, enc[b].T) and its
column/row slice of the weights; returns the partial output transposed
(out_t = (x @ Wo_g)^T, [1024, 2048] f32). Host sums the two head-group
partials per batch and adds the bias.

On-chip design (per core, heads as 4 pairs):
- All matmuls contract over the partition dim; activations arrive transposed
  so no on-chip transposes are needed anywhere.
- Q^T/K^T [512,2048] from projections with W slices as natural lhsT.
- S^T = K_h Q_h^T per head: K=64 contraction; two heads packed concurrently
  in the PE array via tile_position (0,0)/(64,0) writing one [128,1024] f32
  PSUM pair-tile (h1|h2 x 512 queries).
- exp via ScalarE reading the [128,1024] PSUM pair tile in ONE instruction
  (amortizes ACT fixed overhead), scale=1/8 fused, bf16 out (P^T).
- PV: V'[se,64] per head as stationary, col-packed pair via tile_position
  (0,0)/(0,64) accumulating into one [128,512] f32 PSUM tile over 16 se
  chunks. Softmax denominators via ones-vector matmuls col-packed at
  (0,0)/(0,32) into a [33,512] PSUM tile - no reduction passes at all.
- Normalization deferred: reciprocal rows -> broadcast-DMA -> one bf16 DVE
  multiply per pair (doubles as nothing - attn evac is a separate copy).
- Final projection contracts attn^T over the 512 local dims with Wo rows as
  natural lhsT; f32 result DMA'd out.

PSUM budget (8 banks): S pair tiles 2x2 + PV 1 + sums 1 + proj/final 2.
"""

import numpy as np
import ml_dtypes
from contextlib import ExitStack

B = 4
SD = 2048
SE = 2048
E = 1024
H = 16
DH = 64
EL = 512          # local cols per core (8 heads)
NPAIR = 4         # head pairs per core
KCH = E // 128    # embed chunks (8)
SET = SE // 128   # se tiles (16)
SDQ = 512         # sd quarter
NQ = SD // SDQ    # 4
SCALE = 1.0 / np.sqrt(DH)

_BUILT = None


def _build(SD=SD, SE=SE, E=E, SDQ=SDQ, debug=False):
    import concourse.bass as bass
    import concourse.tile as tile
    from concourse import bacc, mybir

    BF16 = mybir.dt.bfloat16
    F32 = mybir.dt.float32
    EXP = mybir.ActivationFunctionType.Exp

    KCH = E // 128
    SET = SE // 128
    NQ = SD // SDQ

    nc = bacc.Bacc("TRN2", target_bir_lowering=False, debug=False)
    dec_t_d = nc.dram_tensor("dec_t", [E, SD], BF16, kind="ExternalInput").ap()
    enc_t_d = nc.dram_tensor("enc_t", [E, SE], BF16, kind="ExternalInput").ap()
    wq_d = nc.dram_tensor("wq", [E, EL], BF16, kind="ExternalInput").ap()
    wk_d = nc.dram_tensor("wk", [E, EL], BF16, kind="ExternalInput").ap()
    wv_d = nc.dram_tensor("wv", [E, EL], BF16, kind="ExternalInput").ap()
    wo_d = nc.dram_tensor("wo", [EL, E], BF16, kind="ExternalInput").ap()
    out_d = nc.dram_tensor("out_t", [E, SD], F32, kind="ExternalOutput").ap()
    recip_dram = nc.dram_tensor("recip_scratch", [4, 2 * SD], BF16).ap()
    dbg = {}
    if debug:
        dbg["qt"] = nc.dram_tensor("dbg_qt", [NPAIR, 128, SD], BF16, kind="ExternalOutput").ap()
        dbg["kt"] = nc.dram_tensor("dbg_kt", [NPAIR, 128, SE], BF16, kind="ExternalOutput").ap()
        dbg["vp"] = nc.dram_tensor("dbg_vp", [SET, 128, 8 * 65], BF16, kind="ExternalOutput").ap()
        dbg["attn"] = nc.dram_tensor("dbg_attn", [NPAIR, 128, SD], BF16, kind="ExternalOutput").ap()
        dbg["recip"] = nc.dram_tensor("dbg_recip", [4, 2 * SD], BF16, kind="ExternalOutput").ap()
        dbg["pt0"] = nc.dram_tensor("dbg_pt0", [128, 2 * SDQ], BF16, kind="ExternalOutput").ap()

    with tile.TileContext(nc) as tc, ExitStack() as ctx:
        consts = ctx.enter_context(tc.tile_pool(name="consts", bufs=1))
        acts = ctx.enter_context(tc.tile_pool(name="acts", bufs=1))
        qk_pool = ctx.enter_context(tc.tile_pool(name="qk", bufs=2))
        v_pool = ctx.enter_context(tc.tile_pool(name="vpool", bufs=1))
        pt_pool = ctx.enter_context(tc.tile_pool(name="pt", bufs=6))
        attn_pool = ctx.enter_context(tc.tile_pool(name="attn", bufs=1))
        small = ctx.enter_context(tc.tile_pool(name="small", bufs=1))
        rbc_pool = ctx.enter_context(tc.tile_pool(name="rbc", bufs=2))
        evac = ctx.enter_context(tc.tile_pool(name="evac", bufs=3))
        ps_s = ctx.enter_context(tc.tile_pool(name="ps_s", bufs=2, space="PSUM"))
        ps_pv = ctx.enter_context(tc.tile_pool(name="ps_pv", bufs=2, space="PSUM"))
        ps_proj = ctx.enter_context(tc.tile_pool(name="ps_proj", bufs=2, space="PSUM"))

        # ---------------- input DMAs ----------------
        wk_sb = consts.tile([128, KCH, EL], BF16, tag="wk", name="wk_sb")
        wq_sb = consts.tile([128, KCH, EL], BF16, tag="wq", name="wq_sb")
        wv_sb = consts.tile([128, KCH, EL], BF16, tag="wv", name="wv_sb")
        enc_sb = [acts.tile([128, SE], BF16, tag=f"enc{k}", name=f"enc{k}")
                  for k in range(KCH)]
        dec_sb = [acts.tile([128, SD], BF16, tag=f"dec{k}", name=f"dec{k}")
                  for k in range(KCH)]
        for k in range(KCH):
            nc.sync.dma_start(out=wk_sb[:, k, :], in_=wk_d[k * 128:(k + 1) * 128, :])
        for k in range(KCH):
            nc.sync.dma_start(out=enc_sb[k][:], in_=enc_t_d[k * 128:(k + 1) * 128, :])
        for k in range(KCH):
            nc.sync.dma_start(out=wq_sb[:, k, :], in_=wq_d[k * 128:(k + 1) * 128, :])
        for k in range(KCH):
            nc.sync.dma_start(out=dec_sb[k][:], in_=dec_t_d[k * 128:(k + 1) * 128, :])
        for k in range(KCH):
            nc.sync.dma_start(out=wv_sb[:, k, :], in_=wv_d[k * 128:(k + 1) * 128, :])
        wo_sb = [consts.tile([128, E], BF16, tag=f"wo{p}", name=f"wo{p}")
                 for p in range(NPAIR)]
        for p in range(NPAIR):
            nc.sync.dma_start(out=wo_sb[p][:], in_=wo_d[p * 128:(p + 1) * 128, :])

        # persistent sbuf tensors
        # V' layout: per head 65 cols = [V_h (64) | ones]; ones column makes
        # the PV matmul emit softmax denominators for free (row 64 of out)
        vp = [v_pool.tile([128, 8 * 65], BF16, tag=f"vp{i}", name=f"vp{i}")
              for i in range(SET)]
        attn_un = [attn_pool.tile([128, SD], BF16, tag=f"attn{p}", name=f"attn{p}")
                   for p in range(NPAIR)]
        # pair p lives on partition 32*p (32-aligned single-partition access),
        # its two heads side by side along the free dim
        recip_all = small.tile([97, 2 * SD], F32, tag="recip", name="recip_all")
        recip_bf = small.tile([97, 2 * SD], BF16, tag="recipbf", name="recip_bf")

        # ---------------- projection helpers ----------------
        def proj_kq(dst, w_sb, src):
            """dst [128, S] bf16 = (W_pair^T @ x^T) for one pair's 128 cols."""
            pair = proj_kq.pair
            for n in range(dst.shape[1] // 512):
                ps = ps_proj.tile([128, 512], F32, tag="ps_proj", name="ps_proj")
                for k in range(KCH):
                    nc.tensor.matmul(
                        ps[:], w_sb[:, k, pair * 128:(pair + 1) * 128],
                        src[k][:, n * 512:(n + 1) * 512],
                        start=(k == 0), stop=(k == KCH - 1))
                nc.vector.tensor_copy(dst[:, n * 512:(n + 1) * 512], ps[:])

        def emit_v():
            for m in range(SET):
                ps = ps_proj.tile([128, 512], F32, tag="ps_proj", name="ps_proj")
                for k in range(KCH):
                    nc.tensor.matmul(
                        ps[:], enc_sb[k][:, m * 128:(m + 1) * 128], wv_sb[:, k, :],
                        start=(k == 0), stop=(k == KCH - 1))
                vsrc = ps[:, :].rearrange("p (h d) -> p h d", h=8)
                vdst = vp[m].rearrange("p (h d) -> p h d", d=65)
                nc.vector.tensor_copy(vdst[:, :, 0:64], vsrc)
                nc.vector.memset(vdst[:, :, 64:65], 1.0)

        # ---------------- attention for one pair ----------------
        def emit_attn(pair, kt, qt):
            for q in range(NQ):
                q0 = q * SDQ
                pvs = [ps_pv.tile([65, SDQ], F32, tag="ps_pv", name=f"ps_pv{h}")
                       for h in range(2)]
                for i in range(SET):
                    sp = ps_s.tile([128, 2 * SDQ], F32, tag="ps_s", name="ps_s")
                    for h in range(2):
                        nc.tensor.matmul(
                            sp[:, h * SDQ:(h + 1) * SDQ],
                            kt[h * 64:(h + 1) * 64, i * 128:(i + 1) * 128],
                            qt[h * 64:(h + 1) * 64, q0:q0 + SDQ],
                            start=True, stop=True,
                            tile_position=(h * 64, 0))
                    pt = pt_pool.tile([128, 2 * SDQ], BF16, tag="pt", name="pt")
                    nc.scalar.activation(pt[:], sp[:], EXP, scale=float(SCALE))
                    if debug and pair == 0 and q == 0 and i == 0:
                        nc.sync.dma_start(out=dbg["pt0"][:, :], in_=pt[:])
                    first, last = (i == 0), (i == SET - 1)
                    for h in range(2):
                        hl = pair * 2 + h
                        nc.tensor.matmul(
                            pvs[h][:, :],
                            vp[i][:, hl * 65:hl * 65 + 65],
                            pt[:, h * SDQ:(h + 1) * SDQ],
                            start=first, stop=last)
                # evacuate: reciprocal of ones-row sums + raw attn copy
                for h in range(2):
                    r = pair * 32
                    nc.vector.reciprocal(
                        recip_all[r:r + 1, h * SD + q0:h * SD + q0 + SDQ],
                        pvs[h][64:65, :])
                    nc.vector.tensor_copy(
                        attn_un[pair][h * 64:(h + 1) * 64, q0:q0 + SDQ],
                        pvs[h][0:64, :])

        # ---------------- emission ----------------
        emit_v()
        if debug:
            for i in range(SET):
                nc.sync.dma_start(out=dbg["vp"][i], in_=vp[i][:])
        for pair in range(NPAIR):
            kt = qk_pool.tile([128, SE], BF16, tag="kt", name="kt")
            qt = qk_pool.tile([128, SD], BF16, tag="qt", name="qt")
            proj_kq.pair = pair
            proj_kq(kt, wk_sb, enc_sb)
            proj_kq(qt, wq_sb, dec_sb)
            if debug:
                nc.sync.dma_start(out=dbg["kt"][pair], in_=kt[:])
                nc.sync.dma_start(out=dbg["qt"][pair], in_=qt[:])
            emit_attn(pair, kt, qt)

        # ---------------- normalization ----------------
        for p in range(NPAIR):
            nc.vector.tensor_copy(recip_bf[32 * p:32 * p + 1, :],
                                  recip_all[32 * p:32 * p + 1, :])
        for p in range(NPAIR):
            nc.sync.dma_start(out=recip_dram[p:p + 1, :],
                              in_=recip_bf[32 * p:32 * p + 1, :])
        for pair in range(NPAIR):
            rbc = rbc_pool.tile([128, SD], BF16, tag="rbc", name="rbc")
            for h in range(2):
                src = recip_dram[pair:pair + 1, h * SD:(h + 1) * SD]
                bcast = bass.AP(tensor=src.tensor, offset=src.offset,
                                ap=[[0, 64], [1, SD]])
                nc.gpsimd.dma_start(out=rbc[h * 64:(h + 1) * 64, :], in_=bcast)
            nc.vector.tensor_mul(attn_un[pair][:], attn_un[pair][:], rbc[:])
            if debug:
                nc.sync.dma_start(out=dbg["attn"][pair], in_=attn_un[pair][:])

        if debug:
            nc.gpsimd.dma_start(out=dbg["recip"][:, :], in_=recip_dram[:, :])
        # ---------------- output projection ----------------
        for m in range(KCH):
            for n in range(SD // 512):
                ps = ps_proj.tile([128, 512], F32, tag="ps_proj", name="ps_proj")
                for p in range(NPAIR):
                    nc.tensor.matmul(
                        ps[:], wo_sb[p][:, m * 128:(m + 1) * 128],
                        attn_un[p][:, n * 512:(n + 1) * 512],
                        start=(p == 0), stop=(p == NPAIR - 1))
                ot = evac.tile([128, 512], F32, tag="ot", name="ot")
                nc.scalar.copy(ot[:], ps[:])
                nc.sync.dma_start(
                    out=out_d[m * 128:(m + 1) * 128, n * 512:(n + 1) * 512],
                    in_=ot[:])

    nc.compile()
    return nc


def _get_built():
    global _BUILT
    if _BUILT is None:
        _BUILT = _build()
    return _BUILT


def kernel(decoder_input, encoder_output, W_q, W_k, W_v, W_o, b_o):
    from concourse.bass_utils import run_bass_kernel_spmd

    dec = np.asarray(decoder_input, dtype=np.float32)
    enc = np.asarray(encoder_output, dtype=np.float32)
    W_q = np.asarray(W_q, dtype=np.float32)
    W_k = np.asarray(W_k, dtype=np.float32)
    W_v = np.asarray(W_v, dtype=np.float32)
    W_o = np.asarray(W_o, dtype=np.float32)
    b_o = np.asarray(b_o, dtype=np.float32)

    bf = lambda a: np.ascontiguousarray(a).astype(ml_dtypes.bfloat16)

    nc = _get_built()
    in_maps = []
    for c in range(8):
        b, g = divmod(c, 2)
        sl = slice(g * EL, (g + 1) * EL)
        in_maps.append({
            "dec_t": bf(dec[b].T),
            "enc_t": bf(enc[b].T),
            "wq": bf(W_q[:, sl]),
            "wk": bf(W_k[:, sl]),
            "wv": bf(W_v[:, sl]),
            "wo": bf(W_o[sl, :]),
        })
    res = run_bass_kernel_spmd(nc, in_maps, core_ids=list(range(8)))
    out = np.empty((B, SD, E), np.float32)
    for b in range(B):
        out[b] = (res.results[2 * b]["out_t"] + res.results[2 * b + 1]["out_t"]).T
        out[b] += b_o
    return out


if __name__ == "__main__":
    _get_built()
    print("kernel built OK")



# revision 7
# speedup vs baseline: 1.2842x; 1.2842x over previous
"""Cross-attention Trainium2 Bass kernel (8 NeuronCores, SPMD).

Problem: B=4, Sd=Se=2048, E=1024, H=16, D=64 cross-attention
  Q = dec @ Wq; K = enc @ Wk; V = enc @ Wv
  out = softmax(Q K^T / sqrt(D)) V @ Wo + b_o

Sharding (hardcoded): core c -> batch b=c//2, head-group g=c%2 (8 heads).
Each core gets transposed bf16 activations (dec[b].T, enc[b].T) and its
column/row slice of the weights; returns the partial output transposed
(out_t = (x @ Wo_g)^T, [1024, 2048] f32). Host sums the two head-group
partials per batch and adds the bias.

On-chip design (per core, heads as 4 pairs):
- All matmuls contract over the partition dim; activations arrive transposed
  so no on-chip transposes are needed anywhere.
- Q^T/K^T [512,2048] from projections with W slices as natural lhsT.
- S^T = K_h Q_h^T per head: K=64 contraction; two heads packed concurrently
  in the PE array via tile_position (0,0)/(64,0) writing one [128,1024] f32
  PSUM pair-tile (h1|h2 x 512 queries).
- exp via ScalarE reading the [128,1024] PSUM pair tile in ONE instruction
  (amortizes ACT fixed overhead), scale=1/8 fused, bf16 out (P^T). The exp
  stream (~270us) is the global bottleneck; everything else hides under it.
- PV: V'[se,64|ones] per head as stationary [128,65]; the ones column emits
  softmax denominators for free as PSUM row 64.
- Normalization q-granular and overlapped: denom rows -> partition-0 staging
  -> DRAM (f32) -> broadcast-DMA to [128,512] -> reciprocal_approx_fast
  (full-partition, base 0 - it silently no-ops at nonzero base partitions!)
  -> bf16 cast -> one multiply per (pair, q-block).
- Final projection contracts attn^T over the 512 local dims with Wo rows as
  natural lhsT; n-groups 0-2 are PE filler inside pair-3's attention slots
  (evacuated on DVE so ScalarE's exp stream is not delayed); only the last
  n-group runs after the final attention slot (evacuated on then-idle ACT).

Scheduling: a static slot schedule keeps TensorE dense (HAM stays warm):
dummy warmup matmuls cover the input-DMA window, V-projection chunks are
interleaved inside the first attention slot, and every later slot carries
2-3 K/Q projection chunks of a future pair (deadline-checked), then
out-projection n-groups.

PSUM budget (8 banks): S pair tiles 2x2 + PV 2x1 + proj 2x1.
"""

import numpy as np
import ml_dtypes
from contextlib import ExitStack

B = 4
SD = 2048
SE = 2048
E = 1024
H = 16
DH = 64
EL = 512          # local cols per core (8 heads)
NPAIR = 4         # head pairs per core
KCH = E // 128    # embed chunks (8)
SET = SE // 128   # se tiles (16)
SDQ = 512         # sd quarter
NQ = SD // SDQ    # 4
SCALE = 1.0 / np.sqrt(DH)

_BUILT = None


def _build(SD=SD, SE=SE, E=E, SDQ=SDQ):
    import concourse.bass as bass
    import concourse.tile as tile
    from concourse import bacc, mybir

    BF16 = mybir.dt.bfloat16
    F32 = mybir.dt.float32
    EXP = mybir.ActivationFunctionType.Exp

    KCH = E // 128
    SET = SE // 128
    NQ = SD // SDQ

    nc = bacc.Bacc("TRN2", target_bir_lowering=False, debug=False)
    dec_t_d = nc.dram_tensor("dec_t", [E, SD], BF16, kind="ExternalInput").ap()
    enc_t_d = nc.dram_tensor("enc_t", [E, SE], BF16, kind="ExternalInput").ap()
    wq_d = nc.dram_tensor("wq", [E, EL], BF16, kind="ExternalInput").ap()
    wk_d = nc.dram_tensor("wk", [E, EL], BF16, kind="ExternalInput").ap()
    wv_d = nc.dram_tensor("wv", [E, EL], BF16, kind="ExternalInput").ap()
    wo_d = nc.dram_tensor("wo", [EL, E], BF16, kind="ExternalInput").ap()
    out_d = nc.dram_tensor("out_t", [E, SD], F32, kind="ExternalOutput").ap()
    recipf_dram = nc.dram_tensor("recipf_scratch", [16, 1024], F32).ap()

    with tile.TileContext(nc) as tc, ExitStack() as ctx:
        consts = ctx.enter_context(tc.tile_pool(name="consts", bufs=1))
        acts = ctx.enter_context(tc.tile_pool(name="acts", bufs=1))
        qk_pool = ctx.enter_context(tc.tile_pool(name="qk", bufs=2))
        v_pool = ctx.enter_context(tc.tile_pool(name="vpool", bufs=1))
        pt_pool = ctx.enter_context(tc.tile_pool(name="pt", bufs=6))
        attn_pool = ctx.enter_context(tc.tile_pool(name="attn", bufs=1))
        small = ctx.enter_context(tc.tile_pool(name="small", bufs=1))
        rbc_pool = ctx.enter_context(tc.tile_pool(name="rbc", bufs=2))
        evac = ctx.enter_context(tc.tile_pool(name="evac", bufs=3))
        ps_s = ctx.enter_context(tc.tile_pool(name="ps_s", bufs=2, space="PSUM"))
        ps_pv = ctx.enter_context(tc.tile_pool(name="ps_pv", bufs=2, space="PSUM"))
        ps_proj = ctx.enter_context(tc.tile_pool(name="ps_proj", bufs=2, space="PSUM"))

        # ---------------- PE warmup ----------------
        # ~16 dummy matmuls fill the input-DMA window so the HAM clock gate
        # opens (1.2 -> 2.4 GHz) before the first real projection issues.
        junk = small.tile([128, 512], BF16, tag="junk", name="junk")
        nc.vector.memset(junk[:], 0.0)
        for w in range(16):
            ps = ps_proj.tile([128, 512], F32, tag="ps_proj", name="ps_warm")
            nc.tensor.matmul(ps[:], junk[:, 0:128], junk[:], start=True, stop=True)

        # ---------------- input DMAs ----------------
        # Ordered so the K projection's inputs (wk, enc) land first; a second
        # queue (scalar/ACT, idle during load) carries the later inputs.
        wk_sb = consts.tile([128, KCH, EL], BF16, tag="wk", name="wk_sb")
        wq_sb = consts.tile([128, KCH, EL], BF16, tag="wq", name="wq_sb")
        wv_sb = consts.tile([128, KCH, EL], BF16, tag="wv", name="wv_sb")
        enc_sb = [acts.tile([128, SE], BF16, tag=f"enc{k}", name=f"enc{k}")
                  for k in range(KCH)]
        dec_sb = [acts.tile([128, SD], BF16, tag=f"dec{k}", name=f"dec{k}")
                  for k in range(KCH)]
        for k in range(KCH):
            nc.sync.dma_start(out=wk_sb[:, k, :], in_=wk_d[k * 128:(k + 1) * 128, :])
        for k in range(KCH):
            nc.sync.dma_start(out=enc_sb[k][:], in_=enc_t_d[k * 128:(k + 1) * 128, :])
        for k in range(KCH):
            nc.scalar.dma_start(out=wv_sb[:, k, :], in_=wv_d[k * 128:(k + 1) * 128, :])
        for k in range(KCH):
            nc.scalar.dma_start(out=wq_sb[:, k, :], in_=wq_d[k * 128:(k + 1) * 128, :])
        for k in range(KCH):
            nc.scalar.dma_start(out=dec_sb[k][:], in_=dec_t_d[k * 128:(k + 1) * 128, :])
        wo_sb = [consts.tile([128, E], BF16, tag=f"wo{p}", name=f"wo{p}")
                 for p in range(NPAIR)]
        for p in range(NPAIR):
            nc.scalar.dma_start(out=wo_sb[p][:], in_=wo_d[p * 128:(p + 1) * 128, :])

        # persistent sbuf tensors
        vp = [v_pool.tile([128, 8 * 65], BF16, tag=f"vp{i}", name=f"vp{i}")
              for i in range(SET)]
        attn_un = [attn_pool.tile([128, SD], BF16, tag=f"attn{p}", name=f"attn{p}")
                   for p in range(NPAIR)]

        # ---------------- chunk-level helpers ----------------
        def proj_chunk(dst, w_sb, pair, src, n):
            """dst[:, n*512:(n+1)*512] = (W_pair^T @ x^T) one 512-col chunk."""
            ps = ps_proj.tile([128, 512], F32, tag="ps_proj", name="ps_proj")
            for k in range(KCH):
                nc.tensor.matmul(
                    ps[:], w_sb[:, k, pair * 128:(pair + 1) * 128],
                    src[k][:, n * 512:(n + 1) * 512],
                    start=(k == 0), stop=(k == KCH - 1))
            nc.vector.tensor_copy(dst[:, n * 512:(n + 1) * 512], ps[:])

        def vproj_chunk(m):
            ps = ps_proj.tile([128, 512], F32, tag="ps_proj", name="ps_proj")
            for k in range(KCH):
                nc.tensor.matmul(
                    ps[:], enc_sb[k][:, m * 128:(m + 1) * 128], wv_sb[:, k, :],
                    start=(k == 0), stop=(k == KCH - 1))
            vsrc = ps[:, :].rearrange("p (h d) -> p h d", h=8)
            vdst = vp[m].rearrange("p (h d) -> p h d", d=65)
            nc.vector.tensor_copy(vdst[:, :, 0:64], vsrc)
            nc.vector.memset(vdst[:, :, 64:65], 1.0)

        def attn_slot(pair, q, kt, qt, fill=None):
            """One (pair, q-block) attention slot; returns the denom tile.

            fill: optional list of SET callables; fill[i]() is emitted after
            chunk i's S/exp so the PE queue carries extra work while the
            ScalarE exp stream grinds.
            """
            q0 = q * SDQ
            pvs = [ps_pv.tile([65, SDQ], F32, tag="ps_pv", name=f"ps_pv{h}")
                   for h in range(2)]
            for i in range(SET):
                sp = ps_s.tile([128, 2 * SDQ], F32, tag="ps_s", name="ps_s")
                for h in range(2):
                    nc.tensor.matmul(
                        sp[:, h * SDQ:(h + 1) * SDQ],
                        kt[h * 64:(h + 1) * 64, i * 128:(i + 1) * 128],
                        qt[h * 64:(h + 1) * 64, q0:q0 + SDQ],
                        start=True, stop=True,
                        tile_position=(h * 64, 0))
                pt = pt_pool.tile([128, 2 * SDQ], BF16, tag="pt", name="pt")
                nc.scalar.activation(pt[:], sp[:], EXP, scale=float(SCALE))
                if fill is not None:
                    fill[i]()
                first, last = (i == 0), (i == SET - 1)
                for h in range(2):
                    hl = pair * 2 + h
                    nc.tensor.matmul(
                        pvs[h][:, :],
                        vp[i][:, hl * 65:hl * 65 + 65],
                        pt[:, h * SDQ:(h + 1) * SDQ],
                        start=first, stop=last)
            # evacuate: denom rows to partition-0 staging + raw attn copy
            dst = small.tile([1, 2 * SDQ], F32, tag="dst", name="dstage", bufs=3)
            for h in range(2):
                nc.vector.tensor_copy(dst[0:1, h * SDQ:(h + 1) * SDQ],
                                      pvs[h][64:65, :])
                nc.vector.tensor_copy(
                    attn_un[pair][h * 64:(h + 1) * 64, q0:q0 + SDQ],
                    pvs[h][0:64, :])
            return dst

        def norm_pq(pair, q, dst):
            """Normalize attn_un[pair][:, q-block]: denom -> DRAM -> bcast ->
            full-partition approx reciprocal -> bf16 -> multiply."""
            q0 = q * SDQ
            row = pair * NQ + q
            nc.sync.dma_start(out=recipf_dram[row:row + 1, :], in_=dst[0:1, :])
            src = recipf_dram[row:row + 1, :]
            bcast = bass.AP(tensor=src.tensor, offset=src.offset,
                            ap=[[SDQ, 2], [0, 64], [1, SDQ]])
            rbc = rbc_pool.tile([128, SDQ], F32, tag="rbc", name="rbc", bufs=3)
            nc.gpsimd.dma_start(out=rbc[:], in_=bcast)
            rbr = rbc_pool.tile([128, SDQ], F32, tag="rbr", name="rbr")
            nc.vector.reciprocal_approx_fast(out=rbr[:], in_=rbc[:])
            rbb = rbc_pool.tile([128, SDQ], BF16, tag="rbb", name="rbb")
            nc.vector.tensor_copy(rbb[:], rbr[:])
            nc.vector.tensor_mul(attn_un[pair][:, q0:q0 + SDQ],
                                 attn_un[pair][:, q0:q0 + SDQ], rbb[:])

        def outproj_chunk(m, n, on_act=False):
            ps = ps_proj.tile([128, 512], F32, tag="ps_proj", name="ps_proj")
            for p in range(NPAIR):
                nc.tensor.matmul(
                    ps[:], wo_sb[p][:, m * 128:(m + 1) * 128],
                    attn_un[p][:, n * 512:(n + 1) * 512],
                    start=(p == 0), stop=(p == NPAIR - 1))
            ot = evac.tile([128, 512], F32, tag="ot", name="ot")
            if on_act:
                nc.scalar.copy(ot[:], ps[:])
            else:
                nc.vector.tensor_copy(ot[:], ps[:])
            nc.sync.dma_start(
                out=out_d[m * 128:(m + 1) * 128, n * 512:(n + 1) * 512],
                in_=ot[:])

        # ---------------- schedule ----------------
        kt_t, qt_t = {}, {}

        def alloc_ktqt(p):
            kt_t[p] = qk_pool.tile([128, SE], BF16, tag="kt", name=f"kt{p}")
            qt_t[p] = qk_pool.tile([128, SD], BF16, tag="qt", name=f"qt{p}")

        def emit_chunk(kind, p, n):
            if p not in kt_t:
                alloc_ktqt(p)
            if kind == "k":
                proj_chunk(kt_t[p], wk_sb, p, enc_sb, n)
            elif kind == "q":
                proj_chunk(qt_t[p], wq_sb, p, dec_sb, n)
            else:  # ("o", m, n): out-projection filler chunk
                outproj_chunk(p, n, on_act=False)

        # startup: K proj pair 0 + first Q chunk; V is interleaved in slot 0
        alloc_ktqt(0)
        for n in range(SE // 512):
            proj_chunk(kt_t[0], wk_sb, 0, enc_sb, n)
        proj_chunk(qt_t[0], wq_sb, 0, dec_sb, 0)

        # per-slot PE filler (deadlines: k/q of pair p before slot (p, n))
        filler = {
            (0, 0): [("q", 0, 1), ("q", 0, 2)],
            (0, 1): [("q", 0, 3), ("k", 1, 0), ("k", 1, 1)],
            (0, 2): [("k", 1, 2), ("k", 1, 3), ("q", 1, 0)],
            (0, 3): [("q", 1, 1), ("q", 1, 2)],
            (1, 0): [("q", 1, 3), ("k", 2, 0)],
            (1, 1): [("k", 2, 1), ("k", 2, 2)],
            (1, 2): [("k", 2, 3), ("q", 2, 0)],
            (1, 3): [("q", 2, 1), ("q", 2, 2)],
            (2, 0): [("q", 2, 3), ("k", 3, 0)],
            (2, 1): [("k", 3, 1), ("k", 3, 2)],
            (2, 2): [("k", 3, 3), ("q", 3, 0)],
            (2, 3): [("q", 3, 1), ("q", 3, 2)],
            (3, 0): [("q", 3, 3)],
            (3, 1): [("o", m, 0) for m in range(KCH)],
            (3, 2): [("o", m, 1) for m in range(KCH)],
            (3, 3): [("o", m, 2) for m in range(KCH)],
        }

        for pair in range(NPAIR):
            for q in range(NQ):
                fill = None
                if pair == 0 and q == 0:
                    fill = [(lambda m=m: vproj_chunk(m)) for m in range(SET)]
                dst = attn_slot(pair, q, kt_t[pair], qt_t[pair], fill=fill)
                norm_pq(pair, q, dst)
                for item in filler.get((pair, q), ()):
                    emit_chunk(*item)

        # tail: last out-projection n-group on the now-idle ScalarE
        for m in range(KCH):
            outproj_chunk(m, SD // 512 - 1, on_act=True)

    nc.compile()
    return nc


def _get_built():
    global _BUILT
    if _BUILT is None:
        _BUILT = _build()
    return _BUILT


def kernel(decoder_input, encoder_output, W_q, W_k, W_v, W_o, b_o):
    from concourse.bass_utils import run_bass_kernel_spmd

    dec = np.asarray(decoder_input, dtype=np.float32)
    enc = np.asarray(encoder_output, dtype=np.float32)
    W_q = np.asarray(W_q, dtype=np.float32)
    W_k = np.asarray(W_k, dtype=np.float32)
    W_v = np.asarray(W_v, dtype=np.float32)
    W_o = np.asarray(W_o, dtype=np.float32)
    b_o = np.asarray(b_o, dtype=np.float32)

    bf = lambda a: np.ascontiguousarray(a).astype(ml_dtypes.bfloat16)

    nc = _get_built()
    in_maps = []
    for c in range(8):
        b, g = divmod(c, 2)
        sl = slice(g * EL, (g + 1) * EL)
        in_maps.append({
            "dec_t": bf(dec[b].T),
            "enc_t": bf(enc[b].T),
            "wq": bf(W_q[:, sl]),
            "wk": bf(W_k[:, sl]),
            "wv": bf(W_v[:, sl]),
            "wo": bf(W_o[sl, :]),
        })
    res = run_bass_kernel_spmd(nc, in_maps, core_ids=list(range(8)))
    out = np.empty((B, SD, E), np.float32)
    for b in range(B):
        out[b] = (res.results[2 * b]["out_t"] + res.results[2 * b + 1]["out_t"]).T
        out[b] += b_o
    return out


if __name__ == "__main__":
    _get_built()
    print("kernel built OK")
